# revision 23
# baseline (speedup 1.0000x reference)
"""Multi-head attention (B=2, S=2048, D=1024, H=16, Dk=64) on 8 NeuronCores.

Sharding: 2-way data parallel over batch x 4-way tensor parallel over heads.
Core c = 4*b + g handles batch b, head group g (4 heads = 256 cols).
W_o is row-sliced; the 4 partial outputs per batch are summed on host (+bo).

fp8(e4m3) DoubleRow design (0.5 cycles/row on PE):
  host: x = x_hi + x_lo (fp8 error-feedback pair), 16*W = w_hi + w_lo,
        16*b = b_hi + b_lo riding the two slots of one aug DR matmul
  Q/K proj: (x_hi+x_lo)@w_hi + x_hi@w_lo + bias-aug -> fp8 q8/k8, then
        full-tensor DMA rearrange into per-head split-dk [32p, 2slot, S]
  scores: 1 fully-packed DR matmul per (head, sk-chunk): psum = 256*q.k
  exp: ACT true-exp->fp8 / DVE Schraudolph int8 bit-trick, alternating
  PV: DR over sk-chunk pairs, v_hi + v_lo feedback; ones-cols give rowsums
  normalize: o16 = (oo/16)*recip(rowsum) -> bf16 (per head, pre-O-proj)
  O-proj: bf16 x bf16 (no fp8 error on the output path)
"""
import math
import numpy as np
import ml_dtypes
from contextlib import ExitStack

import concourse.bass as bass
import concourse.mybir as mybir
import concourse.tile as tile
from concourse import bacc
from concourse.bass_utils import run_bass_kernel_spmd

F8 = mybir.dt.float8e4
F32 = mybir.dt.float32
BF16 = mybir.dt.bfloat16
I8 = mybir.dt.int8
NF8 = ml_dtypes.float8_e4m3
NBF = ml_dtypes.bfloat16
DR = mybir.MatmulPerfMode.DoubleRow
EXP = mybir.ActivationFunctionType.Exp
COPY = mybir.ActivationFunctionType.Copy
MUL = mybir.AluOpType.mult
ADD = mybir.AluOpType.add
SUB = mybir.AluOpType.subtract

B, S, D = 2, 2048, 1024
H, DK = 16, 64
P = 128
KO = 8            # 128-contraction chunks in D
W = 256           # local width (4 heads x 64)
MT = 2            # m-tiles (head pairs)
NQ = 4            # sq chunks of 512
NSK = 16          # sk chunks of 128
SA = 8 * math.log2(math.e) / 2048.0   # Schraudolph scale
SB = 56.0 - 0.48                      # Schraudolph bias (e4m3, RNE convert)

_CACHE = {}


def build_nc():
    nc = bacc.Bacc("TRN2", target_bir_lowering=False, debug=False, num_devices=8)
    xhl = nc.dram_tensor("xhl", [2 * D, S], F8, kind="ExternalInput").ap()
    whlq = nc.dram_tensor("whlq", [2 * D, W], F8, kind="ExternalInput").ap()
    whlk = nc.dram_tensor("whlk", [2 * D, W], F8, kind="ExternalInput").ap()
    whlv = nc.dram_tensor("whlv", [2 * D, W], F8, kind="ExternalInput").ap()
    wob = nc.dram_tensor("wob", [W, D], BF16, kind="ExternalInput").ap()
    # bmall: [bmq(2*2*128) | bmk(2*2*128) | bmv(2*256)] per partition
    bmall = nc.dram_tensor("bmall", [P, 1536], F8, kind="ExternalInput").ap()
    out = nc.dram_tensor("out", [S, D], BF16, kind="ExternalOutput").ap()

    x_r = xhl.rearrange("(t ko p) s -> p (t ko) s", p=P, t=2)
    wq_r = whlq.rearrange("(t ko p) w -> p (t ko) w", p=P, t=2)
    wk_r = whlk.rearrange("(t ko p) w -> p (t ko) w", p=P, t=2)
    wv_r = whlv.rearrange("(t ko p) w -> p (t ko) w", p=P, t=2)
    wob_r = wob.rearrange("(hp p) d -> p hp d", p=P)

    with tile.TileContext(nc) as tc, ExitStack() as ctx:
        sb = ctx.enter_context(tc.tile_pool(name="sb", bufs=1))
        ptp = ctx.enter_context(tc.tile_pool(name="ptp", bufs=4))
        otp = ctx.enter_context(tc.tile_pool(name="otp", bufs=2))
        rcpp = ctx.enter_context(tc.tile_pool(name="rcpp", bufs=2))
        obp = ctx.enter_context(tc.tile_pool(name="obp", bufs=2))
        ps = ctx.enter_context(tc.tile_pool(name="ps", bufs=1, space="PSUM"))

        # ---- resident tiles ----
        x_t = sb.tile([P, 2 * KO, S], F8)      # hi chunks 0:8, lo 8:16
        wq_t = sb.tile([P, 2 * KO, W], F8)
        wk_t = sb.tile([P, 2 * KO, W], F8)
        wv_t = sb.tile([P, 2 * KO, W], F8)
        wob_t = sb.tile([P, MT, D], BF16)
        bm_t = sb.tile([P, 1536], F8)
        bmq_v = bm_t[:, 0:512].rearrange("p (m s c) -> p m s c", m=MT, s=2)
        bmk_v = bm_t[:, 512:1024].rearrange("p (m s c) -> p m s c", m=MT, s=2)
        bmv_v = bm_t[:, 1024:1536].rearrange("p (s c) -> p s c", s=2)
        onex = sb.tile([P, 2, 512], F8)
        qst = sb.tile([P, MT, S], F8)          # quant staging (pre-split)
        kst = sb.tile([P, MT, S], F8)
        # split-dk layout [32*(h%2) + dk%32, dk-half slot, s]; two tiles per
        # tensor because matmul APs may only start at partitions 0/32/64
        qsp = [sb.tile([P, 2, S], F8, name=f"qsp{t}") for t in range(2)]
        ksp = [sb.tile([P, 2, S], F8, name=f"ksp{t}") for t in range(2)]
        vah = sb.tile([P, NSK, 512], F8)       # per head: 64 feat + 64 ones
        val = sb.tile([P, NSK, 512], F8)

        # constants (gpsimd = SBUF-only engine, otherwise idle)
        nc.gpsimd.memset(onex[:], 0.0)
        nc.gpsimd.memset(onex[0:1, :, :], 1.0)   # both slots: bias hi+lo
        nc.gpsimd.memset(val[:], 0.0)
        va4 = vah[:].rearrange("p s (h c) -> p s h c", c=P)
        nc.gpsimd.memset(va4[:, :, :, DK:P], 1.0)

        # loads (SP queue, dependency order: K0/Q0/V00 need wk/x0/wq/wv
        # first; x quarters arrive just-in-time for K1-3)
        nc.sync.dma_start(wk_t[:], wk_r)
        nc.sync.dma_start(x_t[:, 0:KO, 0:512], x_r[:, 0:KO, 0:512])
        nc.sync.dma_start(x_t[:, KO:2 * KO, 0:512], x_r[:, KO:2 * KO, 0:512])
        nc.sync.dma_start(wq_t[:], wq_r)
        nc.sync.dma_start(wv_t[:], wv_r)
        nc.sync.dma_start(bm_t[:], bmall)
        for qtr in range(1, 4):
            sq = slice(qtr * 512, (qtr + 1) * 512)
            nc.sync.dma_start(x_t[:, 0:KO, sq], x_r[:, 0:KO, sq])
            nc.sync.dma_start(x_t[:, KO:2 * KO, sq], x_r[:, KO:2 * KO, sq])
        nc.sync.dma_start(wob_t[:], wob_r)

        # greedy engine balance for exp ops (ns of queued work per engine)
        eng_ns = {"act": 0.0, "dve": 0.0}

        # ---- projection emitters ----
        def proj_qk(dst, qtr):
            wt = wq_t if dst == "q" else wk_t
            bm = bmq_v if dst == "q" else bmk_v
            st = qst if dst == "q" else kst
            sq = slice(qtr * 512, (qtr + 1) * 512)
            pp = ps.tile([P, 1024], F32, tag="sp", bufs=3, name=f"pp_{dst}{qtr}")
            for m in range(MT):
                o = pp[:, m * 512:(m + 1) * 512]
                idx = 0
                for xc, wc in ((0, 0), (KO, 0), (0, KO)):  # hi@hi, lo@hi, hi@lo
                    for kp in range(4):
                        nc.tensor.matmul(
                            o, wt[:, wc + 2 * kp:wc + 2 * kp + 2,
                                  m * P:(m + 1) * P],
                            x_t[:, xc + 2 * kp:xc + 2 * kp + 2, sq],
                            start=(idx == 0), stop=False, perf_mode=DR)
                        idx += 1
                nc.tensor.matmul(o, bm[:, m], onex[:], start=False, stop=True,
                                 perf_mode=DR)
            nc.scalar.activation(
                st[:, :, sq], pp[:].rearrange("p (m c) -> p m c", m=MT),
                COPY, bias=0.0, scale=1.0)
            eng_ns["act"] += 1038

        def rearr(dst, s0=0, s1=S):
            st = qst if dst == "q" else kst
            tgt = qsp if dst == "q" else ksp
            for m in range(MT):
                for j in range(2):
                    for i in range(2):
                        nc.sync.dma_start(
                            tgt[m][32 * j:32 * j + 32, i, s0:s1],
                            st[64 * j + 32 * i:64 * j + 32 * i + 32, m, s0:s1])

        def proj_v(qtr, sp2):
            c0 = qtr * 4 + 2 * sp2
            pv = ps.tile([P, 512], F32, tag="sp", bufs=3, padded_shape=[P, 1024],
                         name=f"pv{qtr}_{sp2}")
            for g in range(2):
                o = pv[:, g * 256:(g + 1) * 256]
                sq = slice(qtr * 512 + (2 * sp2 + g) * P,
                           qtr * 512 + (2 * sp2 + g + 1) * P)
                idx = 0
                for xc, wc in ((0, 0), (KO, 0), (0, KO)):
                    for kp in range(4):
                        nc.tensor.matmul(
                            o, x_t[:, xc + 2 * kp:xc + 2 * kp + 2, sq],
                            wv_t[:, wc + 2 * kp:wc + 2 * kp + 2, :],
                            start=(g == 0 and idx == 0), stop=False,
                            perf_mode=DR, skip_group_check=True)
                        idx += 1
                nc.tensor.matmul(o, onex[:, :, 0:P], bmv_v[:],
                                 start=False, stop=(g == 1), perf_mode=DR,
                                 skip_group_check=True)
            pvr = pv[:].rearrange("p (g h c) -> p g h c", g=2, c=DK)
            vhd = vah[:, c0:c0 + 2, :].rearrange(
                "p s (h c) -> p s h c", c=P)[:, :, :, 0:DK]
            vld = val[:, c0:c0 + 2, :].rearrange(
                "p s (h c) -> p s h c", c=P)[:, :, :, 0:DK]
            nc.scalar.activation(vhd, pvr, COPY, bias=0.0, scale=1.0)
            nc.vector.scalar_tensor_tensor(vld, pvr, 1.0, vhd, MUL, SUB)
            eng_ns["act"] += 611
            eng_ns["dve"] += 658

        # ---- attention ----

        def attn_block(q, hp, drip=None):
            sq = slice(q * 512, (q + 1) * 512)
            oo = ps.tile([P, 2, 512], F32, tag="oo", name=f"oo{q}_{hp}")
            pts = {}

            def emit_pv(p):
                # PV for s-pair p, emitted one pair late so positional
                # (tick-counter) waits match true deps on later scores
                pt = pts.pop(p)
                for j in range(2):
                    h = 2 * hp + j
                    rhs = pt[:, :, j * 512:(j + 1) * 512]
                    nc.tensor.matmul(
                        oo[:, j, :], vah[:, 2 * p:2 * p + 2, h * P:(h + 1) * P],
                        rhs, start=(p == 0), stop=False, perf_mode=DR)
                    nc.tensor.matmul(
                        oo[:, j, :], val[:, 2 * p:2 * p + 2, h * P:(h + 1) * P],
                        rhs, start=False, stop=(p == NSK // 2 - 1),
                        perf_mode=DR)

            for s in range(NSK):
                sp = ps.tile([P, 1024], F32, tag="sp", bufs=3, name=f"sp{q}_{hp}_{s}")
                for j in range(2):
                    h = 2 * hp + j
                    b0 = 32 * (h % 2)
                    nc.tensor.matmul(
                        sp[:, j * 512:(j + 1) * 512],
                        ksp[h // 2][b0:b0 + 32, :, s * P:(s + 1) * P],
                        qsp[h // 2][b0:b0 + 32, :, sq],
                        start=True, stop=True, perf_mode=DR)
                if s % 2 == 0:
                    pts[s // 2] = ptp.tile([P, 2, 1024], F8, tag="pt",
                                           name=f"pt{q}_{hp}_{s}")
                if drip is not None and s < len(drip) and drip[s] is not None:
                    drip[s]()
                pt = pts[s // 2]
                if s >= NSK - 2 or eng_ns["act"] + 1038 <= eng_ns["dve"] + 1192:
                    nc.scalar.activation(pt[:, s % 2, :], sp[:], EXP,
                                         bias=0.0, scale=1.0 / 2048.0)
                    eng_ns["act"] += 1038
                else:
                    nc.vector.tensor_scalar(pt[:].bitcast(I8)[:, s % 2, :],
                                            sp[:], SA, SB, MUL, ADD)
                    eng_ns["dve"] += 1192
                if s % 2 == 1 and s >= 3:
                    emit_pv((s - 3) // 2)
            emit_pv(NSK // 2 - 1)
            rcp = rcpp.tile([64, 2, 512], F32, tag="rcp", name=f"rc{q}_{hp}")
            nc.vector.reciprocal(rcp[:], oo[64:P, :, :])
            eng_ns["dve"] += 1192 + 1316
            return oo, rcp

        def o16_write(ot, oo, rcp, hp):
            for j in range(2):
                nc.vector.scalar_tensor_tensor(
                    ot[64 * j:64 * j + 64, hp, :], oo[0:64, j, :],
                    1.0 / 16.0, rcp[:, j, :], MUL, MUL)

        def o_proj_pieces(q, ot):
            st = {}

            def piece(mo):
                if mo == 0:
                    st['ob'] = obp.tile([P, 4, 2, 512], BF16, tag="ob",
                                        name=f"ob{q}")
                ob = st['ob']
                po = ps.tile([P, 1024], F32, tag="sp", bufs=3,
                             name=f"po{q}_{mo}")
                for n in range(2):
                    for hp2 in range(MT):
                        nc.tensor.matmul(
                            po[:, n * 512:(n + 1) * 512],
                            ot[:, hp2, mo * P:(mo + 1) * P],
                            wob_t[:, hp2, n * 512:(n + 1) * 512],
                            start=(hp2 == 0), stop=(hp2 == MT - 1))
                nc.scalar.activation(
                    ob[:, mo, :, :], po[:].rearrange("p (n c) -> p n c", n=2),
                    COPY, bias=0.0, scale=1.0)
                eng_ns["act"] += 1038
                if mo == 3:
                    dst = out[q * 512:(q + 1) * 512, :].rearrange(
                        "(mo p) (n c) -> p mo n c", p=P, n=2)
                    nc.sync.dma_start(dst, ob[:])

            return [lambda m=mo: piece(m) for mo in range(4)]

        # ---- schedule ----
        proj_qk("k", 0)
        proj_qk("q", 0)
        proj_v(0, 0)
        for qtr in range(1, 4):
            proj_qk("k", qtr)
        rearr("k")
        rearr("q", 0, 512)

        # block (0,0) drip: V chains at even s (chunks 2p+2,2p+3 consumed by
        # the delayed PV at s=2p+5); Q quarters 1-3 at odd s (needed at the
        # next block, s>=16)
        def kq_q(qtr):
            proj_qk("q", qtr)
            rearr("q", qtr * 512, (qtr + 1) * 512)

        _vchains = [lambda a=q2, b=s2: proj_v(a, b)
                    for q2, s2 in ((0, 1), (1, 0), (1, 1), (2, 0),
                                   (2, 1), (3, 0), (3, 1))]
        _qchains = [lambda qt=qtr: kq_q(qt) for qtr in range(1, 4)]
        drip0 = []
        for i, fn in enumerate(_vchains):
            drip0 += [fn, _qchains[i] if i < len(_qchains) else None]

        ot_prev = None
        for q in range(NQ):
            ot = otp.tile([P, MT, 512], BF16, tag="ot", name=f"ot{q}")
            for hp in range(MT):
                if (q, hp) == (0, 0):
                    drip = drip0
                elif hp == 0 and q > 0:
                    drip = o_proj_pieces(q - 1, ot_prev)
                else:
                    drip = None
                oo, rcp = attn_block(q, hp, drip)
                o16_write(ot, oo, rcp, hp)
            ot_prev = ot
        for fn in o_proj_pieces(NQ - 1, ot_prev):
            fn()
    nc.compile()
    return nc


def _f8(x):
    return np.asarray(x, np.float32).astype(NF8)


def _fb(x):
    hi = _f8(x)
    lo = _f8(np.asarray(x, np.float32) - hi.astype(np.float32))
    return hi, lo


def _prep_inputs(x, Wq, bq, Wk, bk, Wv, bv, Wo, bo):
    in_maps = []
    xb = []
    for b in range(B):
        hi, lo = _fb(np.ascontiguousarray(x[b].T))
        xb.append(np.concatenate([hi, lo], axis=0))
    for c in range(8):
        b, g = c // 4, c % 4
        cs = slice(g * W, (g + 1) * W)
        whl = {}
        for n, Wm in (("q", Wq), ("k", Wk), ("v", Wv)):
            hi, lo = _fb(16 * Wm[:, cs])
            whl[n] = np.concatenate([hi, lo], axis=0)
        bmall = np.zeros((P, 1536), NF8)
        for off, bvec in ((0, bq[cs]), (512, bk[cs])):
            bh, bl = _fb(16 * bvec)
            for m in range(MT):
                bmall[0, off + m * 256:off + m * 256 + 128] = bh[m * P:(m + 1) * P]
                bmall[0, off + m * 256 + 128:off + m * 256 + 256] = bl[m * P:(m + 1) * P]
        bh, bl = _fb(16 * bv[cs])
        bmall[0, 1024:1280] = bh
        bmall[0, 1280:1536] = bl
        in_maps.append({
            "xhl": xb[b], "whlq": whl["q"], "whlk": whl["k"], "whlv": whl["v"],
            "wob": np.asarray(Wo[cs, :], np.float32).astype(NBF),
            "bmall": bmall,
        })
    return in_maps


def kernel(x, Wq, bq, Wk, bk, Wv, bv, Wo, bo):
    x = np.asarray(x, dtype=np.float32)
    Wq, bq = np.asarray(Wq, np.float32), np.asarray(bq, np.float32)
    Wk, bk = np.asarray(Wk, np.float32), np.asarray(bk, np.float32)
    Wv, bv = np.asarray(Wv, np.float32), np.asarray(bv, np.float32)
    Wo, bo = np.asarray(Wo, np.float32), np.asarray(bo, np.float32)

    if "nc" not in _CACHE:
        _CACHE["nc"] = build_nc()
    nc = _CACHE["nc"]

    in_maps = _prep_inputs(x, Wq, bq, Wk, bk, Wv, bv, Wo, bo)
    res = run_bass_kernel_spmd(nc, in_maps, core_ids=list(range(8))).results

    out = np.empty((B, S, D), dtype=np.float32)
    for b in range(B):
        acc = res[4 * b]["out"].astype(np.float32)
        for g in range(1, 4):
            acc += res[4 * b + g]["out"].astype(np.float32)
        out[b] = acc + bo
    return out


# revision 24
# speedup vs baseline: 1.0261x; 1.0261x over previous
"""Multi-head attention (B=2, S=2048, D=1024, H=16, Dk=64) on 8 NeuronCores.

Sharding: 2-way data parallel over batch x 4-way tensor parallel over heads.
Core c = 4*b + g handles batch b, head group g (4 heads = 256 cols).
W_o is row-sliced; the 4 partial outputs per batch are summed on host (+bo).

fp8(e4m3) DoubleRow design (0.5 cycles/row on PE):
  host: x = x_hi + x_lo (fp8 error-feedback pair), 16*W = w_hi + w_lo,
        16*b = b_hi + b_lo riding the two slots of one aug DR matmul
  Q/K proj: (x_hi+x_lo)@w_hi + x_hi@w_lo + bias-aug -> fp8 q8/k8, then
        full-tensor DMA rearrange into per-head split-dk [32p, 2slot, S]
  scores: 1 fully-packed DR matmul per (head, sk-chunk): psum = 256*q.k
  exp: ACT true-exp->fp8 / DVE Schraudolph int8 bit-trick, alternating
  PV: DR over sk-chunk pairs, v_hi + v_lo feedback; ones-cols give rowsums
  normalize: o16 = (oo/16)*recip(rowsum) -> bf16 (per head, pre-O-proj)
  O-proj: bf16 x bf16 (no fp8 error on the output path)
"""
import math
import numpy as np
import ml_dtypes
from contextlib import ExitStack

import concourse.bass as bass
import concourse.mybir as mybir
import concourse.tile as tile
from concourse import bacc
from concourse.bass_utils import run_bass_kernel_spmd

F8 = mybir.dt.float8e4
F32 = mybir.dt.float32
BF16 = mybir.dt.bfloat16
I8 = mybir.dt.int8
NF8 = ml_dtypes.float8_e4m3
NBF = ml_dtypes.bfloat16
DR = mybir.MatmulPerfMode.DoubleRow
EXP = mybir.ActivationFunctionType.Exp
COPY = mybir.ActivationFunctionType.Copy
MUL = mybir.AluOpType.mult
ADD = mybir.AluOpType.add
SUB = mybir.AluOpType.subtract

B, S, D = 2, 2048, 1024
H, DK = 16, 64
P = 128
KO = 8            # 128-contraction chunks in D
W = 256           # local width (4 heads x 64)
MT = 2            # m-tiles (head pairs)
NQ = 4            # sq chunks of 512
NSK = 16          # sk chunks of 128
SA = 8 * math.log2(math.e) / 2048.0   # Schraudolph scale
SB = 56.0 - 0.48                      # Schraudolph bias (e4m3, RNE convert)

_CACHE = {}


def build_nc():
    nc = bacc.Bacc("TRN2", target_bir_lowering=False, debug=False, num_devices=8)
    xhl = nc.dram_tensor("xhl", [2 * D, S], F8, kind="ExternalInput").ap()
    whlq = nc.dram_tensor("whlq", [2 * D, W], F8, kind="ExternalInput").ap()
    whlk = nc.dram_tensor("whlk", [2 * D, W], F8, kind="ExternalInput").ap()
    whlv = nc.dram_tensor("whlv", [2 * D, W], F8, kind="ExternalInput").ap()
    wob = nc.dram_tensor("wob", [W, D], BF16, kind="ExternalInput").ap()
    # bmall: [bmq(2*2*128) | bmk(2*2*128) | bmv(2*256)] per partition
    bmall = nc.dram_tensor("bmall", [P, 1536], F8, kind="ExternalInput").ap()
    out = nc.dram_tensor("out", [S, D], BF16, kind="ExternalOutput").ap()

    x_r = xhl.rearrange("(t ko p) s -> p (t ko) s", p=P, t=2)
    wq_r = whlq.rearrange("(t ko p) w -> p (t ko) w", p=P, t=2)
    wk_r = whlk.rearrange("(t ko p) w -> p (t ko) w", p=P, t=2)
    wv_r = whlv.rearrange("(t ko p) w -> p (t ko) w", p=P, t=2)
    wob_r = wob.rearrange("(hp p) d -> p hp d", p=P)

    with tile.TileContext(nc) as tc, ExitStack() as ctx:
        sb = ctx.enter_context(tc.tile_pool(name="sb", bufs=1))
        ptp = ctx.enter_context(tc.tile_pool(name="ptp", bufs=4))
        otp = ctx.enter_context(tc.tile_pool(name="otp", bufs=2))
        rcpp = ctx.enter_context(tc.tile_pool(name="rcpp", bufs=2))
        obp = ctx.enter_context(tc.tile_pool(name="obp", bufs=2))
        ps = ctx.enter_context(tc.tile_pool(name="ps", bufs=1, space="PSUM"))

        # ---- resident tiles ----
        x_t = sb.tile([P, 2 * KO, S], F8)      # hi chunks 0:8, lo 8:16
        wq_t = sb.tile([P, 2 * KO, W], F8)
        wk_t = sb.tile([P, 2 * KO, W], F8)
        wv_t = sb.tile([P, 2 * KO, W], F8)
        wob_t = sb.tile([P, MT, D], BF16)
        bm_t = sb.tile([P, 1536], F8)
        bmq_v = bm_t[:, 0:512].rearrange("p (m s c) -> p m s c", m=MT, s=2)
        bmk_v = bm_t[:, 512:1024].rearrange("p (m s c) -> p m s c", m=MT, s=2)
        bmv_v = bm_t[:, 1024:1536].rearrange("p (s c) -> p s c", s=2)
        onex = sb.tile([P, 2, 512], F8)
        qst = sb.tile([P, MT, S], F8)          # quant staging (pre-split)
        kst = sb.tile([P, MT, S], F8)
        # split-dk layout [32*(h%2) + dk%32, dk-half slot, s]; two tiles per
        # tensor because matmul APs may only start at partitions 0/32/64
        qsp = [sb.tile([P, 2, S], F8, name=f"qsp{t}") for t in range(2)]
        ksp = [sb.tile([P, 2, S], F8, name=f"ksp{t}") for t in range(2)]
        vah = sb.tile([P, NSK, 512], F8)       # per head: 64 feat + 64 ones
        val = sb.tile([P, NSK, 512], F8)

        # constants (gpsimd = SBUF-only engine, otherwise idle)
        nc.gpsimd.memset(onex[:], 0.0)
        nc.gpsimd.memset(onex[0:1, :, :], 1.0)   # both slots: bias hi+lo
        nc.gpsimd.memset(val[:], 0.0)
        va4 = vah[:].rearrange("p s (h c) -> p s h c", c=P)
        nc.gpsimd.memset(va4[:, :, :, DK:P], 1.0)

        # loads (SP queue, dependency order: K0/Q0/V00 need wk/x0/wq/wv
        # first; x quarters arrive just-in-time for K1-3)
        nc.sync.dma_start(wk_t[:], wk_r)
        nc.sync.dma_start(x_t[:, 0:KO, 0:512], x_r[:, 0:KO, 0:512])
        nc.sync.dma_start(x_t[:, KO:2 * KO, 0:512], x_r[:, KO:2 * KO, 0:512])
        nc.sync.dma_start(wq_t[:], wq_r)
        nc.sync.dma_start(wv_t[:], wv_r)
        nc.sync.dma_start(bm_t[:], bmall)
        for qtr in range(1, 4):
            sq = slice(qtr * 512, (qtr + 1) * 512)
            nc.sync.dma_start(x_t[:, 0:KO, sq], x_r[:, 0:KO, sq])
            nc.sync.dma_start(x_t[:, KO:2 * KO, sq], x_r[:, KO:2 * KO, sq])
        nc.sync.dma_start(wob_t[:], wob_r)

        # greedy engine balance for exp ops (ns of queued work per engine)
        eng_ns = {"act": 0.0, "dve": 0.0}

        # ---- projection emitters ----
        def proj_qk(dst, qtr):
            wt = wq_t if dst == "q" else wk_t
            bm = bmq_v if dst == "q" else bmk_v
            st = qst if dst == "q" else kst
            sq = slice(qtr * 512, (qtr + 1) * 512)
            pp = ps.tile([P, 1024], F32, tag="sp", bufs=3, name=f"pp_{dst}{qtr}")
            for m in range(MT):
                o = pp[:, m * 512:(m + 1) * 512]
                idx = 0
                for xc, wc in ((0, 0), (KO, 0), (0, KO)):  # hi@hi, lo@hi, hi@lo
                    for kp in range(4):
                        nc.tensor.matmul(
                            o, wt[:, wc + 2 * kp:wc + 2 * kp + 2,
                                  m * P:(m + 1) * P],
                            x_t[:, xc + 2 * kp:xc + 2 * kp + 2, sq],
                            start=(idx == 0), stop=False, perf_mode=DR)
                        idx += 1
                nc.tensor.matmul(o, bm[:, m], onex[:], start=False, stop=True,
                                 perf_mode=DR)
            nc.scalar.activation(
                st[:, :, sq], pp[:].rearrange("p (m c) -> p m c", m=MT),
                COPY, bias=0.0, scale=1.0)
            eng_ns["act"] += 1038

        def rearr(dst, s0=0, s1=S):
            st = qst if dst == "q" else kst
            tgt = qsp if dst == "q" else ksp
            for m in range(MT):
                for j in range(2):
                    for i in range(2):
                        nc.sync.dma_start(
                            tgt[m][32 * j:32 * j + 32, i, s0:s1],
                            st[64 * j + 32 * i:64 * j + 32 * i + 32, m, s0:s1])

        def proj_v(qtr, sp2):
            c0 = qtr * 4 + 2 * sp2
            pv = ps.tile([P, 512], F32, tag="sp", bufs=3, padded_shape=[P, 1024],
                         name=f"pv{qtr}_{sp2}")
            for g in range(2):
                o = pv[:, g * 256:(g + 1) * 256]
                sq = slice(qtr * 512 + (2 * sp2 + g) * P,
                           qtr * 512 + (2 * sp2 + g + 1) * P)
                idx = 0
                for xc, wc in ((0, 0), (KO, 0), (0, KO)):
                    for kp in range(4):
                        nc.tensor.matmul(
                            o, x_t[:, xc + 2 * kp:xc + 2 * kp + 2, sq],
                            wv_t[:, wc + 2 * kp:wc + 2 * kp + 2, :],
                            start=(g == 0 and idx == 0), stop=False,
                            perf_mode=DR, skip_group_check=True)
                        idx += 1
                nc.tensor.matmul(o, onex[:, :, 0:P], bmv_v[:],
                                 start=False, stop=(g == 1), perf_mode=DR,
                                 skip_group_check=True)
            pvr = pv[:].rearrange("p (g h c) -> p g h c", g=2, c=DK)
            vhd = vah[:, c0:c0 + 2, :].rearrange(
                "p s (h c) -> p s h c", c=P)[:, :, :, 0:DK]
            vld = val[:, c0:c0 + 2, :].rearrange(
                "p s (h c) -> p s h c", c=P)[:, :, :, 0:DK]
            nc.scalar.activation(vhd, pvr, COPY, bias=0.0, scale=1.0)
            nc.vector.scalar_tensor_tensor(vld, pvr, 1.0, vhd, MUL, SUB)
            eng_ns["act"] += 611
            eng_ns["dve"] += 658

        # ---- attention ----

        def attn_block(q, hp, drip=None):
            sq = slice(q * 512, (q + 1) * 512)
            oo = ps.tile([P, 2, 512], F32, tag="oo", name=f"oo{q}_{hp}")
            pts = {}

            def emit_pv(p):
                # PV for s-pair p, emitted one pair late so positional
                # (tick-counter) waits match true deps on later scores
                pt = pts.pop(p)
                for j in range(2):
                    h = 2 * hp + j
                    rhs = pt[:, :, j * 512:(j + 1) * 512]
                    nc.tensor.matmul(
                        oo[:, j, :], vah[:, 2 * p:2 * p + 2, h * P:(h + 1) * P],
                        rhs, start=(p == 0), stop=False, perf_mode=DR)
                    nc.tensor.matmul(
                        oo[:, j, :], val[:, 2 * p:2 * p + 2, h * P:(h + 1) * P],
                        rhs, start=False, stop=(p == NSK // 2 - 1),
                        perf_mode=DR)

            for s in range(NSK):
                sp = ps.tile([P, 1024], F32, tag="sp", bufs=3, name=f"sp{q}_{hp}_{s}")
                for j in range(2):
                    h = 2 * hp + j
                    b0 = 32 * (h % 2)
                    nc.tensor.matmul(
                        sp[:, j * 512:(j + 1) * 512],
                        ksp[h // 2][b0:b0 + 32, :, s * P:(s + 1) * P],
                        qsp[h // 2][b0:b0 + 32, :, sq],
                        start=True, stop=True, perf_mode=DR)
                if s % 2 == 0:
                    pts[s // 2] = ptp.tile([P, 2, 1024], F8, tag="pt",
                                           name=f"pt{q}_{hp}_{s}")
                if drip is not None and s < len(drip) and drip[s] is not None:
                    drip[s]()
                pt = pts[s // 2]
                if s >= NSK - 2 or eng_ns["act"] + 1038 <= eng_ns["dve"] + 1192:
                    nc.scalar.activation(pt[:, s % 2, :], sp[:], EXP,
                                         bias=0.0, scale=1.0 / 2048.0)
                    eng_ns["act"] += 1038
                else:
                    nc.vector.tensor_scalar(pt[:].bitcast(I8)[:, s % 2, :],
                                            sp[:], SA, SB, MUL, ADD)
                    eng_ns["dve"] += 1192
                if s % 2 == 1 and s >= 3:
                    emit_pv((s - 3) // 2)
            emit_pv(NSK // 2 - 1)
            rcp = rcpp.tile([64, 2, 512], F32, tag="rcp", name=f"rc{q}_{hp}")
            nc.vector.reciprocal(rcp[:], oo[64:P, :, :])
            eng_ns["dve"] += 1192 + 1316
            return oo, rcp

        def o16_write(ot, oo, rcp, hp):
            for j in range(2):
                nc.vector.scalar_tensor_tensor(
                    ot[64 * j:64 * j + 64, hp, :], oo[0:64, j, :],
                    1.0 / 16.0, rcp[:, j, :], MUL, MUL)

        def o_proj_pieces(q, ot):
            st = {}

            def piece(mo):
                if mo == 0:
                    st['ob'] = obp.tile([P, 4, 2, 512], BF16, tag="ob",
                                        name=f"ob{q}")
                ob = st['ob']
                po = ps.tile([P, 1024], F32, tag="sp", bufs=3,
                             name=f"po{q}_{mo}")
                for n in range(2):
                    for hp2 in range(MT):
                        nc.tensor.matmul(
                            po[:, n * 512:(n + 1) * 512],
                            ot[:, hp2, mo * P:(mo + 1) * P],
                            wob_t[:, hp2, n * 512:(n + 1) * 512],
                            start=(hp2 == 0), stop=(hp2 == MT - 1))
                nc.scalar.activation(
                    ob[:, mo, :, :], po[:].rearrange("p (n c) -> p n c", n=2),
                    COPY, bias=0.0, scale=1.0)
                eng_ns["act"] += 1038
                if mo == 3:
                    dst = out[q * 512:(q + 1) * 512, :].rearrange(
                        "(mo p) (n c) -> p mo n c", p=P, n=2)
                    nc.sync.dma_start(dst, ob[:])

            return [lambda m=mo: piece(m) for mo in range(4)]

        # ---- schedule ----
        proj_qk("k", 0)
        proj_qk("q", 0)
        proj_v(0, 0)
        for qtr in range(1, 4):
            proj_qk("k", qtr)
        rearr("k")
        rearr("q", 0, 512)
        for qtr in range(1, 4):
            proj_qk("q", qtr)
        rearr("q", 512, S)

        # V-projection chains dripped into block (0,0) at even s-steps; the
        # chain at s-pair p produces sk-chunks 2p+2,2p+3, consumed by the
        # delayed PV at s-pair p+1
        _vchains = [lambda a=q2, b=s2: proj_v(a, b)
                    for q2, s2 in ((0, 1), (1, 0), (1, 1), (2, 0),
                                   (2, 1), (3, 0), (3, 1))]
        drip0 = []
        for fn in _vchains:
            drip0 += [fn, None]

        ot_prev = None
        for q in range(NQ):
            ot = otp.tile([P, MT, 512], BF16, tag="ot", name=f"ot{q}")
            for hp in range(MT):
                if (q, hp) == (0, 0):
                    drip = drip0
                elif hp == 0 and q > 0:
                    drip = o_proj_pieces(q - 1, ot_prev)
                else:
                    drip = None
                oo, rcp = attn_block(q, hp, drip)
                o16_write(ot, oo, rcp, hp)
            ot_prev = ot
        for fn in o_proj_pieces(NQ - 1, ot_prev):
            fn()
    nc.compile()
    return nc


def _f8(x):
    return np.asarray(x, np.float32).astype(NF8)


def _fb(x):
    hi = _f8(x)
    lo = _f8(np.asarray(x, np.float32) - hi.astype(np.float32))
    return hi, lo


def _prep_inputs(x, Wq, bq, Wk, bk, Wv, bv, Wo, bo):
    in_maps = []
    xb = []
    for b in range(B):
        hi, lo = _fb(np.ascontiguousarray(x[b].T))
        xb.append(np.concatenate([hi, lo], axis=0))
    for c in range(8):
        b, g = c // 4, c % 4
        cs = slice(g * W, (g + 1) * W)
        whl = {}
        for n, Wm in (("q", Wq), ("k", Wk), ("v", Wv)):
            hi, lo = _fb(16 * Wm[:, cs])
            whl[n] = np.concatenate([hi, lo], axis=0)
        bmall = np.zeros((P, 1536), NF8)
        for off, bvec in ((0, bq[cs]), (512, bk[cs])):
            bh, bl = _fb(16 * bvec)
            for m in range(MT):
                bmall[0, off + m * 256:off + m * 256 + 128] = bh[m * P:(m + 1) * P]
                bmall[0, off + m * 256 + 128:off + m * 256 + 256] = bl[m * P:(m + 1) * P]
        bh, bl = _fb(16 * bv[cs])
        bmall[0, 1024:1280] = bh
        bmall[0, 1280:1536] = bl
        in_maps.append({
            "xhl": xb[b], "whlq": whl["q"], "whlk": whl["k"], "whlv": whl["v"],
            "wob": np.asarray(Wo[cs, :], np.float32).astype(NBF),
            "bmall": bmall,
        })
    return in_maps


def kernel(x, Wq, bq, Wk, bk, Wv, bv, Wo, bo):
    x = np.asarray(x, dtype=np.float32)
    Wq, bq = np.asarray(Wq, np.float32), np.asarray(bq, np.float32)
    Wk, bk = np.asarray(Wk, np.float32), np.asarray(bk, np.float32)
    Wv, bv = np.asarray(Wv, np.float32), np.asarray(bv, np.float32)
    Wo, bo = np.asarray(Wo, np.float32), np.asarray(bo, np.float32)

    if "nc" not in _CACHE:
        _CACHE["nc"] = build_nc()
    nc = _CACHE["nc"]

    in_maps = _prep_inputs(x, Wq, bq, Wk, bk, Wv, bv, Wo, bo)
    res = run_bass_kernel_spmd(nc, in_maps, core_ids=list(range(8))).results

    out = np.empty((B, S, D), dtype=np.float32)
    for b in range(B):
        acc = res[4 * b]["out"].astype(np.float32)
        for g in range(1, 4):
            acc += res[4 * b + g]["out"].astype(np.float32)
        out[b] = acc + bo
    return out


# revision 25
# speedup vs baseline: 1.0420x; 1.0155x over previous
"""Multi-head attention (B=2, S=2048, D=1024, H=16, Dk=64) on 8 NeuronCores.

Sharding: 2-way data parallel over batch x 4-way tensor parallel over heads.
Core c = 4*b + g handles batch b, head group g (4 heads = 256 cols).
W_o is row-sliced; the 4 partial outputs per batch are summed on host (+bo).

fp8(e4m3) DoubleRow design (0.5 cycles/row on PE):
  host: x = x_hi + x_lo (fp8 error-feedback pair), 16*W = w_hi + w_lo,
        16*b = b_hi + b_lo riding the two slots of one aug DR matmul
  Q/K proj: (x_hi+x_lo)@w_hi + x_hi@w_lo + bias-aug -> fp8 q8/k8, then
        full-tensor DMA rearrange into per-head split-dk [32p, 2slot, S]
  scores: 1 fully-packed DR matmul per (head, sk-chunk): psum = 256*q.k
  exp: ACT true-exp->fp8 / DVE Schraudolph int8 bit-trick, alternating
  PV: DR over sk-chunk pairs, v_hi + v_lo feedback; ones-cols give rowsums
  normalize: o16 = (oo/16)*recip(rowsum) -> bf16 (per head, pre-O-proj)
  O-proj: bf16 x bf16 (no fp8 error on the output path)
"""
import math
import numpy as np
import ml_dtypes
from contextlib import ExitStack

import concourse.bass as bass
import concourse.mybir as mybir
import concourse.tile as tile
from concourse import bacc
from concourse.bass_utils import run_bass_kernel_spmd

F8 = mybir.dt.float8e4
F32 = mybir.dt.float32
BF16 = mybir.dt.bfloat16
I8 = mybir.dt.int8
NF8 = ml_dtypes.float8_e4m3
NBF = ml_dtypes.bfloat16
DR = mybir.MatmulPerfMode.DoubleRow
EXP = mybir.ActivationFunctionType.Exp
COPY = mybir.ActivationFunctionType.Copy
MUL = mybir.AluOpType.mult
ADD = mybir.AluOpType.add
SUB = mybir.AluOpType.subtract

B, S, D = 2, 2048, 1024
H, DK = 16, 64
P = 128
KO = 8            # 128-contraction chunks in D
W = 256           # local width (4 heads x 64)
MT = 2            # m-tiles (head pairs)
NQ = 4            # sq chunks of 512
NSK = 16          # sk chunks of 128
SA = 8 * math.log2(math.e) / 2048.0   # Schraudolph scale
SB = 56.0 - 0.48                      # Schraudolph bias (e4m3, RNE convert)

_CACHE = {}


def build_nc():
    nc = bacc.Bacc("TRN2", target_bir_lowering=False, debug=False, num_devices=8)
    xhl = nc.dram_tensor("xhl", [2 * D, S], F8, kind="ExternalInput").ap()
    whlq = nc.dram_tensor("whlq", [2 * D, W], F8, kind="ExternalInput").ap()
    whlk = nc.dram_tensor("whlk", [2 * D, W], F8, kind="ExternalInput").ap()
    whlv = nc.dram_tensor("whlv", [2 * D, W], F8, kind="ExternalInput").ap()
    wob = nc.dram_tensor("wob", [W, D], BF16, kind="ExternalInput").ap()
    # bmall: [bmq(2*2*128) | bmk(2*2*128) | bmv(2*256)] per partition
    bmall = nc.dram_tensor("bmall", [P, 1536], F8, kind="ExternalInput").ap()
    out = nc.dram_tensor("out", [S, D], BF16, kind="ExternalOutput").ap()

    x_r = xhl.rearrange("(t ko p) s -> p (t ko) s", p=P, t=2)
    wq_r = whlq.rearrange("(t ko p) w -> p (t ko) w", p=P, t=2)
    wk_r = whlk.rearrange("(t ko p) w -> p (t ko) w", p=P, t=2)
    wv_r = whlv.rearrange("(t ko p) w -> p (t ko) w", p=P, t=2)
    wob_r = wob.rearrange("(hp p) d -> p hp d", p=P)

    with tile.TileContext(nc) as tc, ExitStack() as ctx:
        sb = ctx.enter_context(tc.tile_pool(name="sb", bufs=1))
        ptp = ctx.enter_context(tc.tile_pool(name="ptp", bufs=4))
        otp = ctx.enter_context(tc.tile_pool(name="otp", bufs=2))
        rcpp = ctx.enter_context(tc.tile_pool(name="rcpp", bufs=2))
        obp = ctx.enter_context(tc.tile_pool(name="obp", bufs=2))
        ps = ctx.enter_context(tc.tile_pool(name="ps", bufs=1, space="PSUM"))

        # ---- resident tiles ----
        x_t = sb.tile([P, 2 * KO, S], F8)      # hi chunks 0:8, lo 8:16
        wq_t = sb.tile([P, 2 * KO, W], F8)
        wk_t = sb.tile([P, 2 * KO, W], F8)
        wv_t = sb.tile([P, 2 * KO, W], F8)
        wob_t = sb.tile([P, MT, D], BF16)
        bm_t = sb.tile([P, 1536], F8)
        bmq_v = bm_t[:, 0:512].rearrange("p (m s c) -> p m s c", m=MT, s=2)
        bmk_v = bm_t[:, 512:1024].rearrange("p (m s c) -> p m s c", m=MT, s=2)
        bmv_v = bm_t[:, 1024:1536].rearrange("p (s c) -> p s c", s=2)
        onex = sb.tile([P, 2, 512], F8)
        qst = sb.tile([P, MT, S], F8)          # quant staging (pre-split)
        kst = sb.tile([P, MT, S], F8)
        # split-dk layout [32*(h%2) + dk%32, dk-half slot, s]; two tiles per
        # tensor because matmul APs may only start at partitions 0/32/64
        qsp = [sb.tile([P, 2, S], F8, name=f"qsp{t}") for t in range(2)]
        ksp = [sb.tile([P, 2, S], F8, name=f"ksp{t}") for t in range(2)]
        vah = sb.tile([P, NSK, 512], F8)       # per head: 64 feat + 64 ones
        val = sb.tile([P, NSK, 512], F8)

        # constants (gpsimd = SBUF-only engine, otherwise idle)
        nc.gpsimd.memset(onex[:], 0.0)
        nc.gpsimd.memset(onex[0:1, :, :], 1.0)   # both slots: bias hi+lo
        nc.gpsimd.memset(val[:], 0.0)
        va4 = vah[:].rearrange("p s (h c) -> p s h c", c=P)
        nc.gpsimd.memset(va4[:, :, :, DK:P], 1.0)

        # loads (SP queue, dependency order: K0/Q0/V00 need wk/x0/wq/wv
        # first; x quarters arrive just-in-time for K1-3)
        nc.sync.dma_start(wk_t[:], wk_r)
        nc.sync.dma_start(x_t[:, 0:KO, 0:512], x_r[:, 0:KO, 0:512])
        nc.sync.dma_start(x_t[:, KO:2 * KO, 0:512], x_r[:, KO:2 * KO, 0:512])
        nc.sync.dma_start(wq_t[:], wq_r)
        nc.sync.dma_start(wv_t[:], wv_r)
        nc.sync.dma_start(bm_t[:], bmall)
        for qtr in range(1, 4):
            sq = slice(qtr * 512, (qtr + 1) * 512)
            nc.sync.dma_start(x_t[:, 0:KO, sq], x_r[:, 0:KO, sq])
            nc.sync.dma_start(x_t[:, KO:2 * KO, sq], x_r[:, KO:2 * KO, sq])
        nc.sync.dma_start(wob_t[:], wob_r)

        # greedy engine balance for exp ops (ns of queued work per engine)
        eng_ns = {"act": 0.0, "dve": 0.0}

        # ---- projection emitters ----
        def proj_qk(dst, qtr):
            wt = wq_t if dst == "q" else wk_t
            bm = bmq_v if dst == "q" else bmk_v
            st = qst if dst == "q" else kst
            sq = slice(qtr * 512, (qtr + 1) * 512)
            pp = ps.tile([P, 1024], F32, tag="sp", bufs=3, name=f"pp_{dst}{qtr}")
            for m in range(MT):
                o = pp[:, m * 512:(m + 1) * 512]
                idx = 0
                for xc, wc in ((0, 0), (KO, 0), (0, KO)):  # hi@hi, lo@hi, hi@lo
                    for kp in range(4):
                        nc.tensor.matmul(
                            o, wt[:, wc + 2 * kp:wc + 2 * kp + 2,
                                  m * P:(m + 1) * P],
                            x_t[:, xc + 2 * kp:xc + 2 * kp + 2, sq],
                            start=(idx == 0), stop=False, perf_mode=DR)
                        idx += 1
                nc.tensor.matmul(o, bm[:, m], onex[:], start=False, stop=True,
                                 perf_mode=DR)
            nc.scalar.activation(
                st[:, :, sq], pp[:].rearrange("p (m c) -> p m c", m=MT),
                COPY, bias=0.0, scale=1.0)
            eng_ns["act"] += 1038

        def rearr(dst, s0=0, s1=S):
            st = qst if dst == "q" else kst
            tgt = qsp if dst == "q" else ksp
            for m in range(MT):
                for j in range(2):
                    for i in range(2):
                        nc.sync.dma_start(
                            tgt[m][32 * j:32 * j + 32, i, s0:s1],
                            st[64 * j + 32 * i:64 * j + 32 * i + 32, m, s0:s1])

        def proj_v(qtr, sp2):
            c0 = qtr * 4 + 2 * sp2
            pv = ps.tile([P, 512], F32, tag="sp", bufs=3, padded_shape=[P, 1024],
                         name=f"pv{qtr}_{sp2}")
            for g in range(2):
                o = pv[:, g * 256:(g + 1) * 256]
                sq = slice(qtr * 512 + (2 * sp2 + g) * P,
                           qtr * 512 + (2 * sp2 + g + 1) * P)
                idx = 0
                for xc, wc in ((0, 0), (KO, 0), (0, KO)):
                    for kp in range(4):
                        nc.tensor.matmul(
                            o, x_t[:, xc + 2 * kp:xc + 2 * kp + 2, sq],
                            wv_t[:, wc + 2 * kp:wc + 2 * kp + 2, :],
                            start=(g == 0 and idx == 0), stop=False,
                            perf_mode=DR, skip_group_check=True)
                        idx += 1
                nc.tensor.matmul(o, onex[:, :, 0:P], bmv_v[:],
                                 start=False, stop=(g == 1), perf_mode=DR,
                                 skip_group_check=True)
            pvr = pv[:].rearrange("p (g h c) -> p g h c", g=2, c=DK)
            vhd = vah[:, c0:c0 + 2, :].rearrange(
                "p s (h c) -> p s h c", c=P)[:, :, :, 0:DK]
            vld = val[:, c0:c0 + 2, :].rearrange(
                "p s (h c) -> p s h c", c=P)[:, :, :, 0:DK]
            nc.scalar.activation(vhd, pvr, COPY, bias=0.0, scale=1.0)
            nc.vector.scalar_tensor_tensor(vld, pvr, 1.0, vhd, MUL, SUB)
            eng_ns["act"] += 611
            eng_ns["dve"] += 658

        # ---- attention ----

        def attn_block(q, hp, drip=None):
            sq = slice(q * 512, (q + 1) * 512)
            oo = ps.tile([P, 2, 512], F32, tag="oo", name=f"oo{q}_{hp}")
            pts = {}

            def emit_pv(p):
                # PV for s-pair p, emitted one pair late so positional
                # (tick-counter) waits match true deps on later scores
                pt = pts.pop(p)
                for j in range(2):
                    h = 2 * hp + j
                    rhs = pt[:, :, j * 512:(j + 1) * 512]
                    nc.tensor.matmul(
                        oo[:, j, :], vah[:, 2 * p:2 * p + 2, h * P:(h + 1) * P],
                        rhs, start=(p == 0), stop=False, perf_mode=DR)
                    nc.tensor.matmul(
                        oo[:, j, :], val[:, 2 * p:2 * p + 2, h * P:(h + 1) * P],
                        rhs, start=False, stop=(p == NSK // 2 - 1),
                        perf_mode=DR)

            for s in range(NSK):
                sp = ps.tile([P, 1024], F32, tag="sp", bufs=3, name=f"sp{q}_{hp}_{s}")
                for j in range(2):
                    h = 2 * hp + j
                    b0 = 32 * (h % 2)
                    nc.tensor.matmul(
                        sp[:, j * 512:(j + 1) * 512],
                        ksp[h // 2][b0:b0 + 32, :, s * P:(s + 1) * P],
                        qsp[h // 2][b0:b0 + 32, :, sq],
                        start=True, stop=True, perf_mode=DR)
                if s % 2 == 0:
                    pts[s // 2] = ptp.tile([P, 2, 1024], F8, tag="pt",
                                           name=f"pt{q}_{hp}_{s}")
                if drip is not None and s < len(drip) and drip[s] is not None:
                    drip[s]()
                pt = pts[s // 2]
                if s == NSK - 1 or eng_ns["act"] + 1038 <= eng_ns["dve"] + 1192:
                    nc.scalar.activation(pt[:, s % 2, :], sp[:], EXP,
                                         bias=0.0, scale=1.0 / 2048.0)
                    eng_ns["act"] += 1038
                else:
                    nc.vector.tensor_scalar(pt[:].bitcast(I8)[:, s % 2, :],
                                            sp[:], SA, SB, MUL, ADD)
                    eng_ns["dve"] += 1192
                if s % 2 == 1 and s >= 3:
                    emit_pv((s - 3) // 2)
            emit_pv(NSK // 2 - 1)
            rcp = rcpp.tile([64, 2, 512], F32, tag="rcp", name=f"rc{q}_{hp}")
            nc.vector.reciprocal(rcp[:], oo[64:P, :, :])
            eng_ns["dve"] += 1192 + 1316
            return oo, rcp

        def o16_write(ot, oo, rcp, hp):
            for j in range(2):
                nc.vector.scalar_tensor_tensor(
                    ot[64 * j:64 * j + 64, hp, :], oo[0:64, j, :],
                    1.0 / 16.0, rcp[:, j, :], MUL, MUL)

        def o_proj_pieces(q, ot):
            st = {}

            def piece(mo):
                if mo == 0:
                    st['ob'] = obp.tile([P, 4, 2, 512], BF16, tag="ob",
                                        name=f"ob{q}")
                ob = st['ob']
                po = ps.tile([P, 1024], F32, tag="sp", bufs=3,
                             name=f"po{q}_{mo}")
                for n in range(2):
                    for hp2 in range(MT):
                        nc.tensor.matmul(
                            po[:, n * 512:(n + 1) * 512],
                            ot[:, hp2, mo * P:(mo + 1) * P],
                            wob_t[:, hp2, n * 512:(n + 1) * 512],
                            start=(hp2 == 0), stop=(hp2 == MT - 1))
                nc.scalar.activation(
                    ob[:, mo, :, :], po[:].rearrange("p (n c) -> p n c", n=2),
                    COPY, bias=0.0, scale=1.0)
                eng_ns["act"] += 1038
                if mo == 3:
                    dst = out[q * 512:(q + 1) * 512, :].rearrange(
                        "(mo p) (n c) -> p mo n c", p=P, n=2)
                    nc.sync.dma_start(dst, ob[:])

            return [lambda m=mo: piece(m) for mo in range(4)]

        # ---- schedule ----
        proj_qk("k", 0)
        proj_qk("q", 0)
        proj_v(0, 0)
        for qtr in range(1, 4):
            proj_qk("k", qtr)
        rearr("k")
        rearr("q", 0, 512)
        for qtr in range(1, 4):
            proj_qk("q", qtr)
        rearr("q", 512, S)

        # V-projection chains dripped into block (0,0) at even s-steps; the
        # chain at s-pair p produces sk-chunks 2p+2,2p+3, consumed by the
        # delayed PV at s-pair p+1
        _vchains = [lambda a=q2, b=s2: proj_v(a, b)
                    for q2, s2 in ((0, 1), (1, 0), (1, 1), (2, 0),
                                   (2, 1), (3, 0), (3, 1))]
        drip0 = []
        for fn in _vchains:
            drip0 += [fn, None]

        ot_prev = None
        for q in range(NQ):
            ot = otp.tile([P, MT, 512], BF16, tag="ot", name=f"ot{q}")
            for hp in range(MT):
                if (q, hp) == (0, 0):
                    drip = drip0
                elif hp == 0 and q > 0:
                    drip = o_proj_pieces(q - 1, ot_prev)
                else:
                    drip = None
                oo, rcp = attn_block(q, hp, drip)
                o16_write(ot, oo, rcp, hp)
            ot_prev = ot
        for fn in o_proj_pieces(NQ - 1, ot_prev):
            fn()
    nc.compile()
    return nc


def _f8(x):
    return np.asarray(x, np.float32).astype(NF8)


def _fb(x):
    hi = _f8(x)
    lo = _f8(np.asarray(x, np.float32) - hi.astype(np.float32))
    return hi, lo


def _prep_inputs(x, Wq, bq, Wk, bk, Wv, bv, Wo, bo):
    in_maps = []
    xb = []
    for b in range(B):
        hi, lo = _fb(np.ascontiguousarray(x[b].T))
        xb.append(np.concatenate([hi, lo], axis=0))
    for c in range(8):
        b, g = c // 4, c % 4
        cs = slice(g * W, (g + 1) * W)
        whl = {}
        for n, Wm in (("q", Wq), ("k", Wk), ("v", Wv)):
            hi, lo = _fb(16 * Wm[:, cs])
            whl[n] = np.concatenate([hi, lo], axis=0)
        bmall = np.zeros((P, 1536), NF8)
        for off, bvec in ((0, bq[cs]), (512, bk[cs])):
            bh, bl = _fb(16 * bvec)
            for m in range(MT):
                bmall[0, off + m * 256:off + m * 256 + 128] = bh[m * P:(m + 1) * P]
                bmall[0, off + m * 256 + 128:off + m * 256 + 256] = bl[m * P:(m + 1) * P]
        bh, bl = _fb(16 * bv[cs])
        bmall[0, 1024:1280] = bh
        bmall[0, 1280:1536] = bl
        in_maps.append({
            "xhl": xb[b], "whlq": whl["q"], "whlk": whl["k"], "whlv": whl["v"],
            "wob": np.asarray(Wo[cs, :], np.float32).astype(NBF),
            "bmall": bmall,
        })
    return in_maps


def kernel(x, Wq, bq, Wk, bk, Wv, bv, Wo, bo):
    x = np.asarray(x, dtype=np.float32)
    Wq, bq = np.asarray(Wq, np.float32), np.asarray(bq, np.float32)
    Wk, bk = np.asarray(Wk, np.float32), np.asarray(bk, np.float32)
    Wv, bv = np.asarray(Wv, np.float32), np.asarray(bv, np.float32)
    Wo, bo = np.asarray(Wo, np.float32), np.asarray(bo, np.float32)

    if "nc" not in _CACHE:
        _CACHE["nc"] = build_nc()
    nc = _CACHE["nc"]

    in_maps = _prep_inputs(x, Wq, bq, Wk, bk, Wv, bv, Wo, bo)
    res = run_bass_kernel_spmd(nc, in_maps, core_ids=list(range(8))).results

    out = np.empty((B, S, D), dtype=np.float32)
    for b in range(B):
        acc = res[4 * b]["out"].astype(np.float32)
        for g in range(1, 4):
            acc += res[4 * b + g]["out"].astype(np.float32)
        out[b] = acc + bo
    return out


# revision 26
# speedup vs baseline: 1.0460x; 1.0039x over previous
"""Multi-head attention (B=2, S=2048, D=1024, H=16, Dk=64) on 8 NeuronCores.

Sharding: 2-way data parallel over batch x 4-way tensor parallel over heads.
Core c = 4*b + g handles batch b, head group g (4 heads = 256 cols).
W_o is row-sliced; the 4 partial outputs per batch are summed on host (+bo).

fp8(e4m3) DoubleRow design (0.5 cycles/row on PE):
  host: x = x_hi + x_lo (fp8 error-feedback pair), 16*W = w_hi + w_lo,
        16*b = b_hi + b_lo riding the two slots of one aug DR matmul
  Q/K proj: (x_hi+x_lo)@w_hi + x_hi@w_lo + bias-aug -> fp8 q8/k8, then
        full-tensor DMA rearrange into per-head split-dk [32p, 2slot, S]
  scores: 1 fully-packed DR matmul per (head, sk-chunk): psum = 256*q.k
  exp: ACT true-exp->fp8 / DVE Schraudolph int8 bit-trick, alternating
  PV: DR over sk-chunk pairs, v_hi + v_lo feedback; ones-cols give rowsums
  normalize: o16 = (oo/16)*recip(rowsum) -> bf16 (per head, pre-O-proj)
  O-proj: bf16 x bf16 (no fp8 error on the output path)
"""
import math
import numpy as np
import ml_dtypes
from contextlib import ExitStack

import concourse.bass as bass
import concourse.mybir as mybir
import concourse.tile as tile
from concourse import bacc
from concourse.bass_utils import run_bass_kernel_spmd

F8 = mybir.dt.float8e4
F32 = mybir.dt.float32
BF16 = mybir.dt.bfloat16
I8 = mybir.dt.int8
NF8 = ml_dtypes.float8_e4m3
NBF = ml_dtypes.bfloat16
DR = mybir.MatmulPerfMode.DoubleRow
EXP = mybir.ActivationFunctionType.Exp
COPY = mybir.ActivationFunctionType.Copy
MUL = mybir.AluOpType.mult
ADD = mybir.AluOpType.add
SUB = mybir.AluOpType.subtract

B, S, D = 2, 2048, 1024
H, DK = 16, 64
P = 128
KO = 8            # 128-contraction chunks in D
W = 256           # local width (4 heads x 64)
MT = 2            # m-tiles (head pairs)
NQ = 4            # sq chunks of 512
NSK = 16          # sk chunks of 128
SA = 8 * math.log2(math.e) / 2048.0   # Schraudolph scale
SB = 56.0 - 0.48                      # Schraudolph bias (e4m3, RNE convert)

_CACHE = {}


def build_nc():
    nc = bacc.Bacc("TRN2", target_bir_lowering=False, debug=False, num_devices=8)
    xhl = nc.dram_tensor("xhl", [2 * D, S], F8, kind="ExternalInput").ap()
    whlq = nc.dram_tensor("whlq", [2 * D, W], F8, kind="ExternalInput").ap()
    whlk = nc.dram_tensor("whlk", [2 * D, W], F8, kind="ExternalInput").ap()
    whlv = nc.dram_tensor("whlv", [2 * D, W], F8, kind="ExternalInput").ap()
    wob = nc.dram_tensor("wob", [W, D], BF16, kind="ExternalInput").ap()
    # bmall: [bmq(2*2*128) | bmk(2*2*128) | bmv(2*256)] per partition
    bmall = nc.dram_tensor("bmall", [P, 1536], F8, kind="ExternalInput").ap()
    out = nc.dram_tensor("out", [S, D], BF16, kind="ExternalOutput").ap()

    x_r = xhl.rearrange("(t ko p) s -> p (t ko) s", p=P, t=2)
    wq_r = whlq.rearrange("(t ko p) w -> p (t ko) w", p=P, t=2)
    wk_r = whlk.rearrange("(t ko p) w -> p (t ko) w", p=P, t=2)
    wv_r = whlv.rearrange("(t ko p) w -> p (t ko) w", p=P, t=2)
    wob_r = wob.rearrange("(hp p) d -> p hp d", p=P)

    with tile.TileContext(nc) as tc, ExitStack() as ctx:
        sb = ctx.enter_context(tc.tile_pool(name="sb", bufs=1))
        ptp = ctx.enter_context(tc.tile_pool(name="ptp", bufs=4))
        otp = ctx.enter_context(tc.tile_pool(name="otp", bufs=2))
        rcpp = ctx.enter_context(tc.tile_pool(name="rcpp", bufs=2))
        obp = ctx.enter_context(tc.tile_pool(name="obp", bufs=2))
        ps = ctx.enter_context(tc.tile_pool(name="ps", bufs=1, space="PSUM"))

        # ---- resident tiles ----
        x_t = sb.tile([P, 2 * KO, S], F8)      # hi chunks 0:8, lo 8:16
        wq_t = sb.tile([P, 2 * KO, W], F8)
        wk_t = sb.tile([P, 2 * KO, W], F8)
        wv_t = sb.tile([P, 2 * KO, W], F8)
        wob_t = sb.tile([P, MT, D], BF16)
        bm_t = sb.tile([P, 1536], F8)
        bmq_v = bm_t[:, 0:512].rearrange("p (m s c) -> p m s c", m=MT, s=2)
        bmk_v = bm_t[:, 512:1024].rearrange("p (m s c) -> p m s c", m=MT, s=2)
        bmv_v = bm_t[:, 1024:1536].rearrange("p (s c) -> p s c", s=2)
        onex = sb.tile([P, 2, 512], F8)
        qst = sb.tile([P, MT, S], F8)          # quant staging (pre-split)
        kst = sb.tile([P, MT, S], F8)
        # split-dk layout [32*(h%2) + dk%32, dk-half slot, s]; two tiles per
        # tensor because matmul APs may only start at partitions 0/32/64
        qsp = [sb.tile([P, 2, S], F8, name=f"qsp{t}") for t in range(2)]
        ksp = [sb.tile([P, 2, S], F8, name=f"ksp{t}") for t in range(2)]
        vah = sb.tile([P, NSK, 512], F8)       # per head: 64 feat + 64 ones
        val = sb.tile([P, NSK, 512], F8)

        # constants (gpsimd = SBUF-only engine, otherwise idle)
        nc.gpsimd.memset(onex[:], 0.0)
        nc.gpsimd.memset(onex[0:1, :, :], 1.0)   # both slots: bias hi+lo
        nc.gpsimd.memset(val[:], 0.0)
        va4 = vah[:].rearrange("p s (h c) -> p s h c", c=P)
        nc.gpsimd.memset(va4[:, :, :, DK:P], 1.0)

        # loads (SP queue, dependency order: K0/Q0/V00 need wk/x0/wq/wv
        # first; x quarters arrive just-in-time for K1-3)
        nc.sync.dma_start(wk_t[:], wk_r)
        nc.sync.dma_start(x_t[:, 0:KO, 0:512], x_r[:, 0:KO, 0:512])
        nc.sync.dma_start(x_t[:, KO:2 * KO, 0:512], x_r[:, KO:2 * KO, 0:512])
        nc.sync.dma_start(wq_t[:], wq_r)
        nc.sync.dma_start(wv_t[:], wv_r)
        nc.sync.dma_start(bm_t[:], bmall)
        for qtr in range(1, 4):
            sq = slice(qtr * 512, (qtr + 1) * 512)
            nc.sync.dma_start(x_t[:, 0:KO, sq], x_r[:, 0:KO, sq])
            nc.sync.dma_start(x_t[:, KO:2 * KO, sq], x_r[:, KO:2 * KO, sq])
        nc.sync.dma_start(wob_t[:], wob_r)

        # greedy engine balance for exp ops (ns of queued work per engine)
        eng_ns = {"act": 0.0, "dve": 0.0}

        # ---- projection emitters ----
        def proj_qk(dst, qtr):
            wt = wq_t if dst == "q" else wk_t
            bm = bmq_v if dst == "q" else bmk_v
            st = qst if dst == "q" else kst
            sq = slice(qtr * 512, (qtr + 1) * 512)
            pp = ps.tile([P, 1024], F32, tag="sp", bufs=3, name=f"pp_{dst}{qtr}")
            for m in range(MT):
                o = pp[:, m * 512:(m + 1) * 512]
                idx = 0
                for xc, wc in ((0, 0), (KO, 0), (0, KO)):  # hi@hi, lo@hi, hi@lo
                    for kp in range(4):
                        nc.tensor.matmul(
                            o, wt[:, wc + 2 * kp:wc + 2 * kp + 2,
                                  m * P:(m + 1) * P],
                            x_t[:, xc + 2 * kp:xc + 2 * kp + 2, sq],
                            start=(idx == 0), stop=False, perf_mode=DR)
                        idx += 1
                nc.tensor.matmul(o, bm[:, m], onex[:], start=False, stop=True,
                                 perf_mode=DR)
            nc.scalar.activation(
                st[:, :, sq], pp[:].rearrange("p (m c) -> p m c", m=MT),
                COPY, bias=0.0, scale=1.0)
            eng_ns["act"] += 1038

        def rearr(dst, s0=0, s1=S):
            st = qst if dst == "q" else kst
            tgt = qsp if dst == "q" else ksp
            for m in range(MT):
                for j in range(2):
                    for i in range(2):
                        nc.sync.dma_start(
                            tgt[m][32 * j:32 * j + 32, i, s0:s1],
                            st[64 * j + 32 * i:64 * j + 32 * i + 32, m, s0:s1])

        def proj_v(qtr, sp2):
            c0 = qtr * 4 + 2 * sp2
            pv = ps.tile([P, 512], F32, tag="sp", bufs=3, padded_shape=[P, 1024],
                         name=f"pv{qtr}_{sp2}")
            for g in range(2):
                o = pv[:, g * 256:(g + 1) * 256]
                sq = slice(qtr * 512 + (2 * sp2 + g) * P,
                           qtr * 512 + (2 * sp2 + g + 1) * P)
                idx = 0
                for xc, wc in ((0, 0), (KO, 0), (0, KO)):
                    for kp in range(4):
                        nc.tensor.matmul(
                            o, x_t[:, xc + 2 * kp:xc + 2 * kp + 2, sq],
                            wv_t[:, wc + 2 * kp:wc + 2 * kp + 2, :],
                            start=(g == 0 and idx == 0), stop=False,
                            perf_mode=DR, skip_group_check=True)
                        idx += 1
                nc.tensor.matmul(o, onex[:, :, 0:P], bmv_v[:],
                                 start=False, stop=(g == 1), perf_mode=DR,
                                 skip_group_check=True)
            pvr = pv[:].rearrange("p (g h c) -> p g h c", g=2, c=DK)
            vhd = vah[:, c0:c0 + 2, :].rearrange(
                "p s (h c) -> p s h c", c=P)[:, :, :, 0:DK]
            vld = val[:, c0:c0 + 2, :].rearrange(
                "p s (h c) -> p s h c", c=P)[:, :, :, 0:DK]
            nc.scalar.activation(vhd, pvr, COPY, bias=0.0, scale=1.0)
            nc.vector.scalar_tensor_tensor(vld, pvr, 1.0, vhd, MUL, SUB)
            eng_ns["act"] += 611
            eng_ns["dve"] += 658

        # ---- attention ----

        def attn_block(q, hp, drip=None):
            sq = slice(q * 512, (q + 1) * 512)
            oo = ps.tile([P, 2, 512], F32, tag="oo", name=f"oo{q}_{hp}")
            pts = {}

            def emit_pv(p):
                # PV for s-pair p, emitted one pair late so positional
                # (tick-counter) waits match true deps on later scores
                pt = pts.pop(p)
                for j in range(2):
                    h = 2 * hp + j
                    rhs = pt[:, :, j * 512:(j + 1) * 512]
                    nc.tensor.matmul(
                        oo[:, j, :], vah[:, 2 * p:2 * p + 2, h * P:(h + 1) * P],
                        rhs, start=(p == 0), stop=False, perf_mode=DR)
                    nc.tensor.matmul(
                        oo[:, j, :], val[:, 2 * p:2 * p + 2, h * P:(h + 1) * P],
                        rhs, start=False, stop=(p == NSK // 2 - 1),
                        perf_mode=DR)

            for s in range(NSK):
                sp = ps.tile([P, 1024], F32, tag="sp", bufs=3, name=f"sp{q}_{hp}_{s}")
                for j in range(2):
                    h = 2 * hp + j
                    b0 = 32 * (h % 2)
                    nc.tensor.matmul(
                        sp[:, j * 512:(j + 1) * 512],
                        ksp[h // 2][b0:b0 + 32, :, s * P:(s + 1) * P],
                        qsp[h // 2][b0:b0 + 32, :, sq],
                        start=True, stop=True, perf_mode=DR)
                if s % 2 == 0:
                    pts[s // 2] = ptp.tile([P, 2, 1024], F8, tag="pt",
                                           name=f"pt{q}_{hp}_{s}")
                if drip is not None and s < len(drip) and drip[s] is not None:
                    drip[s]()
                pt = pts[s // 2]
                if eng_ns["act"] + 1038 <= eng_ns["dve"] + 1192:
                    nc.scalar.activation(pt[:, s % 2, :], sp[:], EXP,
                                         bias=0.0, scale=1.0 / 2048.0)
                    eng_ns["act"] += 1038
                else:
                    nc.vector.tensor_scalar(pt[:].bitcast(I8)[:, s % 2, :],
                                            sp[:], SA, SB, MUL, ADD)
                    eng_ns["dve"] += 1192
                if s % 2 == 1 and s >= 3:
                    emit_pv((s - 3) // 2)
            emit_pv(NSK // 2 - 1)
            rcp = rcpp.tile([64, 2, 512], F32, tag="rcp", name=f"rc{q}_{hp}")
            nc.vector.reciprocal(rcp[:], oo[64:P, :, :])
            eng_ns["dve"] += 1192 + 1316
            return oo, rcp

        def o16_write(ot, oo, rcp, hp):
            for j in range(2):
                nc.vector.scalar_tensor_tensor(
                    ot[64 * j:64 * j + 64, hp, :], oo[0:64, j, :],
                    1.0 / 16.0, rcp[:, j, :], MUL, MUL)

        def o_proj_pieces(q, ot):
            st = {}

            def piece(mo):
                if mo == 0:
                    st['ob'] = obp.tile([P, 4, 2, 512], BF16, tag="ob",
                                        name=f"ob{q}")
                ob = st['ob']
                po = ps.tile([P, 1024], F32, tag="sp", bufs=3,
                             name=f"po{q}_{mo}")
                for n in range(2):
                    for hp2 in range(MT):
                        nc.tensor.matmul(
                            po[:, n * 512:(n + 1) * 512],
                            ot[:, hp2, mo * P:(mo + 1) * P],
                            wob_t[:, hp2, n * 512:(n + 1) * 512],
                            start=(hp2 == 0), stop=(hp2 == MT - 1))
                nc.scalar.activation(
                    ob[:, mo, :, :], po[:].rearrange("p (n c) -> p n c", n=2),
                    COPY, bias=0.0, scale=1.0)
                eng_ns["act"] += 1038
                if mo == 3:
                    dst = out[q * 512:(q + 1) * 512, :].rearrange(
                        "(mo p) (n c) -> p mo n c", p=P, n=2)
                    nc.sync.dma_start(dst, ob[:])

            return [lambda m=mo: piece(m) for mo in range(4)]

        # ---- schedule ----
        proj_qk("k", 0)
        proj_qk("q", 0)
        proj_v(0, 0)
        for qtr in range(1, 4):
            proj_qk("k", qtr)
        rearr("k")
        rearr("q", 0, 512)
        for qtr in range(1, 4):
            proj_qk("q", qtr)
        rearr("q", 512, S)

        # V-projection chains dripped into block (0,0) at even s-steps; the
        # chain at s-pair p produces sk-chunks 2p+2,2p+3, consumed by the
        # delayed PV at s-pair p+1
        _vchains = [lambda a=q2, b=s2: proj_v(a, b)
                    for q2, s2 in ((0, 1), (1, 0), (1, 1), (2, 0),
                                   (2, 1), (3, 0), (3, 1))]
        drip0 = []
        for fn in _vchains:
            drip0 += [fn, None]

        ot_prev = None
        for q in range(NQ):
            ot = otp.tile([P, MT, 512], BF16, tag="ot", name=f"ot{q}")
            for hp in range(MT):
                if (q, hp) == (0, 0):
                    drip = drip0
                elif hp == 0 and q > 0:
                    drip = o_proj_pieces(q - 1, ot_prev)
                else:
                    drip = None
                oo, rcp = attn_block(q, hp, drip)
                o16_write(ot, oo, rcp, hp)
            ot_prev = ot
        for fn in o_proj_pieces(NQ - 1, ot_prev):
            fn()
    nc.compile()
    return nc


def _f8(x):
    return np.asarray(x, np.float32).astype(NF8)


def _fb(x):
    hi = _f8(x)
    lo = _f8(np.asarray(x, np.float32) - hi.astype(np.float32))
    return hi, lo


def _prep_inputs(x, Wq, bq, Wk, bk, Wv, bv, Wo, bo):
    in_maps = []
    xb = []
    for b in range(B):
        hi, lo = _fb(np.ascontiguousarray(x[b].T))
        xb.append(np.concatenate([hi, lo], axis=0))
    for c in range(8):
        b, g = c // 4, c % 4
        cs = slice(g * W, (g + 1) * W)
        whl = {}
        for n, Wm in (("q", Wq), ("k", Wk), ("v", Wv)):
            hi, lo = _fb(16 * Wm[:, cs])
            whl[n] = np.concatenate([hi, lo], axis=0)
        bmall = np.zeros((P, 1536), NF8)
        for off, bvec in ((0, bq[cs]), (512, bk[cs])):
            bh, bl = _fb(16 * bvec)
            for m in range(MT):
                bmall[0, off + m * 256:off + m * 256 + 128] = bh[m * P:(m + 1) * P]
                bmall[0, off + m * 256 + 128:off + m * 256 + 256] = bl[m * P:(m + 1) * P]
        bh, bl = _fb(16 * bv[cs])
        bmall[0, 1024:1280] = bh
        bmall[0, 1280:1536] = bl
        in_maps.append({
            "xhl": xb[b], "whlq": whl["q"], "whlk": whl["k"], "whlv": whl["v"],
            "wob": np.asarray(Wo[cs, :], np.float32).astype(NBF),
            "bmall": bmall,
        })
    return in_maps


def kernel(x, Wq, bq, Wk, bk, Wv, bv, Wo, bo):
    x = np.asarray(x, dtype=np.float32)
    Wq, bq = np.asarray(Wq, np.float32), np.asarray(bq, np.float32)
    Wk, bk = np.asarray(Wk, np.float32), np.asarray(bk, np.float32)
    Wv, bv = np.asarray(Wv, np.float32), np.asarray(bv, np.float32)
    Wo, bo = np.asarray(Wo, np.float32), np.asarray(bo, np.float32)

    if "nc" not in _CACHE:
        _CACHE["nc"] = build_nc()
    nc = _CACHE["nc"]

    in_maps = _prep_inputs(x, Wq, bq, Wk, bk, Wv, bv, Wo, bo)
    res = run_bass_kernel_spmd(nc, in_maps, core_ids=list(range(8))).results

    out = np.empty((B, S, D), dtype=np.float32)
    for b in range(B):
        acc = res[4 * b]["out"].astype(np.float32)
        for g in range(1, 4):
            acc += res[4 * b + g]["out"].astype(np.float32)
        out[b] = acc + bo
    return out
